# revision 1
# baseline (speedup 1.0000x reference)
"""Trainium2 Bass kernel for a single-head causal attention block.

Computes, per batch b:
    xn    = LayerNorm(x[b])           (non-affine, eps=1e-6)
    q,k,v = xn @ Wq, xn @ Wk, xn @ Wv
    s     = causal_mask(q @ k.T / sqrt(D))
    out   = softmax(s) @ v @ Wo + x[b]

Sharding (8 cores, SPMD single program):
  core c -> batch b = c//4, query stripe j = c%4 (queries {4k+j}).
  Each core computes full-sequence K,V for its batch (duplicated across
  the 4 cores of a batch - no collectives needed), q for its own 1024
  strided queries, causal attention for those queries, then the output
  projection + residual for its own rows.  The strided query assignment
  makes the causal prefix work identical across cores, so one uniform
  program load-balances perfectly.

Matmul orientations (out[M,N] = lhsT[K,M].T @ rhs[K,N], K contracted):
  kT  [d, tok] : lhsT = Wk[h, d] tile,   rhs = xnT[h, tok]
  v   [tok, d] : lhsT = xnT[h, tok],     rhs = Wv[h, d]
  qT  [d, tq]  : lhsT = Wq[h, d] tile,   rhs = xqnT[h, tq]   (scaled 1/sqrt(D))
  sT  [tk, tq] : lhsT = kT[d, tk] tile,  rhs = qT[d, tq]     (scores transposed)
  oaT [d, tq]  : lhsT = v[tk, d] tile,   rhs = pT[tk, tq]    (p = exp(sT+mask))
  out [tq, h]  : lhsT = oaT[d, tq] tile, rhs = Wo[d, h]
Softmax denominators come from an all-ones stationary matmul over pT
(accumulated in PSUM, broadcast across all 128 partitions); the
normalization is folded into the PSUM->SBUF copy of oaT.  exp() is
applied without max subtraction (scores are O(10) here; exp stays far
below fp32 overflow), which matches softmax exactly after normalization.

x_norm is stored token-major in DRAM and transposed into h-partition
layout with DMA crossbar transposes (2-byte dtype) when loaded back in
the projection phase - no PE/DVE transpose work at all.
"""

import numpy as np
import ml_dtypes

import concourse.bacc as bacc
import concourse.tile as tile
from concourse import mybir
from concourse.bass_utils import run_bass_kernel_spmd

# Problem shape (hardcoded per harness contract)
B, S, H, D = 2, 4096, 2048, 2048
NCORES = 8
P = 128                 # partitions
TQ = 512                # query group width (matmul moving dim)


def _derived():
    GQ = NCORES // B     # cores per batch (query stripes)
    SQ = S // GQ         # queries per core
    NGRP = SQ // TQ      # query groups per core
    HT = H // P          # h tiles
    DT = D // P          # d tiles
    NDIAG = TQ * GQ // P  # diagonal (mask) key tiles per query group
    ND8 = D // 2 // P    # PSUM banks per d-half in M3
    return GQ, SQ, NGRP, HT, DT, NDIAG, ND8

F32 = mybir.dt.float32
BF16 = mybir.dt.bfloat16
CDT = BF16              # compute dtype for matmul operands
CDT_NP = ml_dtypes.bfloat16

EPS = 1e-6
NEG = -1e30


def build_nc(compile=True, reps=1):
    GQ, SQ, NGRP, HT, DT, NDIAG, ND8 = _derived()
    nc = bacc.Bacc(num_devices=NCORES)

    # I/O
    xkv = nc.dram_tensor("xkv", [S // GQ, H], F32, kind="ExternalInput")
    xq = nc.dram_tensor("xq", [SQ, H], F32, kind="ExternalInput")
    wq = nc.dram_tensor("wq", [H, D], CDT, kind="ExternalInput")
    wk = nc.dram_tensor("wk", [H, D], CDT, kind="ExternalInput")
    wv = nc.dram_tensor("wv", [H, D], CDT, kind="ExternalInput")
    wo = nc.dram_tensor("wo", [D, H], CDT, kind="ExternalInput")
    # additive causal masks for the 16 diagonal key tiles of a query group
    masks = nc.dram_tensor("masks", [NDIAG, P, TQ], F32, kind="ExternalInput")
    out = nc.dram_tensor("out", [SQ, H], F32, kind="ExternalOutput")

    # DRAM scratch
    SKV = S // GQ  # tokens whose K/V this core projects
    xn_d = nc.dram_tensor("xn_d", [SKV, H], CDT)   # normalized own block
    xqn_d = nc.dram_tensor("xqn_d", [SQ, H], CDT)  # normalized query rows
    qt_d = nc.dram_tensor("qt_d", [D, SQ], CDT)    # Q^T / sqrt(D)
    # K and V side by side so one AllGather moves both (bigger transfer =
    # better collective bandwidth, one launch instead of two serialized)
    kv_own = nc.dram_tensor("kv_own", [SKV, 2 * D], CDT)
    kv_full = nc.dram_tensor("kv_full", [S, 2 * D], CDT)
    CC_GROUPS = [
        list(range(g * GQ, (g + 1) * GQ)) for g in range(NCORES // GQ)
    ]

    with (
        tile.TileContext(nc, pool_alloc_mode="queue") as tc,
        tc.tile_pool(name="consts", bufs=1) as consts,
        # Weights stay resident through their phase; the 16 tags recycle
        # the same slots for Wk -> Wv -> Wq -> Wo in turn.
        tc.tile_pool(name="wpool", bufs=1) as wpool,
    ):
        ones = consts.tile([P, P], CDT)
        nc.vector.memset(ones, 1.0)
        eps_tile = consts.tile([P, 1], F32)
        nc.vector.memset(eps_tile, EPS)

        def load_w(w_dram, n2):
            tiles = []
            for a in range(HT):
                t = wpool.tile([P, n2], CDT, tag=f"w{a}")
                nc.sync.dma_start(out=t, in_=w_dram[a * P : (a + 1) * P, :])
                tiles.append(t)
            return tiles

        for _rep in range(reps):
            NB = S // TQ  # token blocks in xn

            def load_xnT(pool, src_d, tb):
                """Load a [H, TQ] block of x_norm^T via DMA crossbar transpose."""
                tiles = []
                for a in range(HT):
                    t = pool.tile([P, TQ], CDT, tag=f"xt{a}")
                    nc.scalar.dma_start_transpose(
                        t, src_d[tb * TQ : (tb + 1) * TQ, a * P : (a + 1) * P]
                    )
                    tiles.append(t)
                return tiles

            # -------- Phase 1: LayerNorm (fused per block) + projections ----
            with (
                tc.tile_pool(name="xpool", bufs=5) as xpool,
                tc.tile_pool(name="xnpool", bufs=5) as xnpool,
                tc.tile_pool(name="stats", bufs=5) as stats_p,
                tc.tile_pool(name="small", bufs=8) as small_p,
                tc.tile_pool(name="xnT", bufs=2) as xnT_p,
                tc.tile_pool(name="stage1", bufs=4) as stage_p,
                tc.tile_pool(name="pp1", bufs=3, space="PSUM") as pp1,
            ):
                def ln_block(src, dst, tb):
                    """LayerNorm tokens [tb*TQ, (tb+1)*TQ) of src -> dst."""
                    for t in range(tb * TQ // P, (tb + 1) * TQ // P):
                        x_t = xpool.tile([P, H], F32, tag="x")
                        nc.sync.dma_start(out=x_t, in_=src[t * P : (t + 1) * P, :])
                        stats = stats_p.tile([P, H // 512, 6], F32, tag="st")
                        for i in range(H // 512):
                            nc.vector.bn_stats(
                                out=stats[:, i, :],
                                in_=x_t[:, i * 512 : (i + 1) * 512],
                            )
                        mv = small_p.tile([P, 2], F32, tag="mv")
                        nc.vector.bn_aggr(out=mv, in_=stats)
                        # rstd = 1/sqrt(var + eps)
                        sq = small_p.tile([P, 1], F32, tag="sq")
                        nc.scalar.activation(
                            out=sq, in_=mv[:, 1:2],
                            func=mybir.ActivationFunctionType.Sqrt,
                            bias=eps_tile, scale=1.0,
                        )
                        rs = small_p.tile([P, 1], F32, tag="rs")
                        nc.vector.reciprocal(out=rs, in_=sq)
                        xn_t = xnpool.tile([P, H], CDT, tag="xn")
                        nc.vector.tensor_scalar(
                            out=xn_t, in0=x_t, scalar1=mv[:, 0:1], scalar2=rs,
                            op0=mybir.AluOpType.subtract, op1=mybir.AluOpType.mult,
                        )
                        nc.sync.dma_start(
                            out=dst[t * P : (t + 1) * P, :], in_=xn_t
                        )

                # K (natural layout, own token block only; LayerNorm fused)
                wk_sb = load_w(wk, D)
                for tb in range(SKV // TQ):
                    ln_block(xkv, xn_d, tb)
                    xt = load_xnT(xnT_p, xn_d, tb)
                    for tl in range(TQ // P):
                        for dc in range(D // TQ):
                            ps = pp1.tile([P, TQ], F32, tag="ps")
                            for a in range(HT):
                                nc.tensor.matmul(
                                    ps,
                                    xt[a][:, tl * P : (tl + 1) * P],
                                    wk_sb[a][:, dc * TQ : (dc + 1) * TQ],
                                    start=(a == 0), stop=(a == HT - 1),
                                )
                            st = stage_p.tile([P, TQ], CDT, tag="st")
                            nc.any.tensor_copy(st, ps)
                            nc.sync.dma_start(
                                out=kv_own[
                                    tb * TQ + tl * P : tb * TQ + (tl + 1) * P,
                                    dc * TQ : (dc + 1) * TQ,
                                ],
                                in_=st,
                            )
                # V (natural layout, own token block only)
                wv_sb = load_w(wv, D)
                for tb in range(SKV // TQ):
                    xt = load_xnT(xnT_p, xn_d, tb)
                    for tl in range(TQ // P):
                        for dc in range(D // TQ):
                            ps = pp1.tile([P, TQ], F32, tag="ps")
                            for a in range(HT):
                                nc.tensor.matmul(
                                    ps,
                                    xt[a][:, tl * P : (tl + 1) * P],
                                    wv_sb[a][:, dc * TQ : (dc + 1) * TQ],
                                    start=(a == 0), stop=(a == HT - 1),
                                )
                            st = stage_p.tile([P, TQ], CDT, tag="st")
                            nc.any.tensor_copy(st, ps)
                            nc.sync.dma_start(
                                out=kv_own[
                                    tb * TQ + tl * P : tb * TQ + (tl + 1) * P,
                                    D + dc * TQ : D + (dc + 1) * TQ,
                                ],
                                in_=st,
                            )

                nc.gpsimd.collective_compute(
                    "AllGather", mybir.AluOpType.bypass,
                    replica_groups=CC_GROUPS,
                    ins=[kv_own[:, :]], outs=[kv_full[:, :]],
                )
                # Q^T (scaled by 1/sqrt(D)); runs while the KV AllGather is in flight
                wq_sb = load_w(wq, D)
                for tb in range(SQ // TQ):
                    ln_block(xq, xqn_d, tb)
                    xt = load_xnT(xnT_p, xqn_d, tb)
                    for d in range(DT):
                        ps = pp1.tile([P, TQ], F32, tag="ps")
                        for a in range(HT):
                            nc.tensor.matmul(
                                ps, wq_sb[a][:, d * P : (d + 1) * P], xt[a],
                                start=(a == 0), stop=(a == HT - 1),
                            )
                        st = stage_p.tile([P, TQ], CDT, tag="st")
                        nc.scalar.activation(
                            out=st, in_=ps, func=mybir.ActivationFunctionType.Copy,
                            bias=0.0, scale=float(1.0 / np.sqrt(D)),
                        )
                        nc.sync.dma_start(
                            out=qt_d[d * P : (d + 1) * P, tb * TQ : (tb + 1) * TQ],
                            in_=st,
                        )
            # ---------------- Phases 2-4 ----------------
            # Both score passes (need only k_full) run first so the PE stays
            # busy while the V AllGather is still in flight; then P@V and the
            # output projection per query group.
            wo_sb = load_w(wo, H)  # reuses wpool slots once Wq drains

            with (
                tc.tile_pool(name="qg", bufs=1) as qg_p,
                tc.tile_pool(name="ktc", bufs=1) as ktc_p,
                tc.tile_pool(name="pT", bufs=1) as pT_p,
                tc.tile_pool(name="mask", bufs=2) as mask_p,
                tc.tile_pool(name="rec", bufs=1) as rec_p,
                tc.tile_pool(name="oaT", bufs=1) as oaT_p,
            ):
                pTs, recs = [], []
                for g in range(NGRP):
                    TK = (g + 1) * S // NGRP // P  # visible key tiles
                    qg = qg_p.tile([P, DT, TQ], CDT, tag="qg", name=f"qg{g}")
                    nc.sync.dma_start(
                        out=qg,
                        in_=qt_d[:, g * TQ : (g + 1) * TQ].rearrange(
                            "(a p) t -> p a t", p=P
                        ),
                    )
                    pT = pT_p.tile([P, TK, TQ], CDT, tag=f"pT{g}", name=f"pT{g}")
                    rec = rec_p.tile([P, TQ], F32, tag=f"rec{g}", name=f"rec{g}")
                    pTs.append(pT)
                    recs.append(rec)

                    # ----- M2: scores^T, mask, exp, denominators -----
                    with (
                        tc.tile_pool(name="psc", bufs=2, space="PSUM") as psc,
                        tc.tile_pool(name="psums", bufs=1, space="PSUM") as psm,
                    ):
                        sums = psm.tile([P, TQ], F32, tag="sums", name=f"sums{g}")
                        for kc in range(TK // 4):
                            # 4 key tiles per fetch, transposed on load
                            kts = []
                            for a in range(DT):
                                kt_t = ktc_p.tile(
                                    [P, 4 * P], CDT, tag=f"kt{a}",
                                    name=f"kt{g}_{kc}_{a}",
                                )
                                nc.scalar.dma_start_transpose(
                                    kt_t,
                                    kv_full[
                                        kc * 4 * P : (kc + 1) * 4 * P,
                                        a * P : (a + 1) * P,
                                    ],
                                )
                                kts.append(kt_t)
                            for t4 in range(4):
                                tk = kc * 4 + t4
                                ps = psc.tile([P, TQ], F32, tag="ps")
                                for d in range(DT):
                                    nc.tensor.matmul(
                                        ps,
                                        kts[d][:, t4 * P : (t4 + 1) * P],
                                        qg[:, d, :],
                                        start=(d == 0), stop=(d == DT - 1),
                                    )
                                u = tk - (TK - NDIAG)
                                if u >= 0:  # diagonal region: causal mask
                                    mt = mask_p.tile([P, TQ], F32, tag="mt")
                                    nc.sync.dma_start(out=mt, in_=masks[u, :, :])
                                    nc.vector.tensor_add(out=ps, in0=ps, in1=mt)
                                nc.scalar.activation(
                                    out=pT[:, tk, :], in_=ps,
                                    func=mybir.ActivationFunctionType.Exp,
                                )
                                nc.tensor.matmul(
                                    sums, ones, pT[:, tk, :],
                                    start=(tk == 0), stop=(tk == TK - 1),
                                    skip_group_check=True,
                                )
                        nc.vector.reciprocal(out=rec, in_=sums)

                for g in range(NGRP):
                    TK = (g + 1) * S // NGRP // P
                    pT, rec = pTs[g], recs[g]

                    # ----- M3: out_attn^T (normalization folded into copy) --
                    with (
                        tc.tile_pool(name="vst", bufs=3) as vst_p,
                        tc.tile_pool(name="poa", bufs=1, space="PSUM") as poa,
                    ):
                        oaT = oaT_p.tile([P, DT, TQ], CDT, tag="oaT", name=f"oaT{g}")
                        for dh in range(2):
                            pss = []
                            for i in range(ND8):
                                pt = poa.tile(
                                    [P, TQ], F32, tag=f"poa{i}",
                                    name=f"poa{dh}_{i}_{g}",
                                )
                                pss.append(pt)
                            for tk in range(TK):
                                vt = vst_p.tile([P, D // 2], CDT, tag="vt")
                                nc.sync.dma_start(
                                    out=vt,
                                    in_=kv_full[
                                        tk * P : (tk + 1) * P,
                                        D + dh * (D // 2) : D + (dh + 1) * (D // 2),
                                    ],
                                )
                                for d8 in range(ND8):
                                    nc.tensor.matmul(
                                        pss[d8],
                                        vt[:, d8 * P : (d8 + 1) * P],
                                        pT[:, tk, :],
                                        start=(tk == 0), stop=(tk == TK - 1),
                                        skip_group_check=True,
                                    )
                            for d8 in range(ND8):
                                nc.vector.tensor_mul(
                                    out=oaT[:, dh * ND8 + d8, :],
                                    in0=pss[d8], in1=rec,
                                )

                    # ----- M4: output projection + residual -----
                    with (
                        tc.tile_pool(name="res", bufs=2) as res_p,
                        tc.tile_pool(name="ost", bufs=2) as ost_p,
                        tc.tile_pool(name="pfin", bufs=2, space="PSUM") as pfin,
                    ):
                        for tq4 in range(TQ // P):
                            row0 = g * TQ + tq4 * P
                            for hc in range(H // TQ):
                                ps = pfin.tile([P, TQ], F32, tag="ps")
                                for d in range(DT):
                                    nc.tensor.matmul(
                                        ps,
                                        oaT[:, d, tq4 * P : (tq4 + 1) * P],
                                        wo_sb[d][:, hc * TQ : (hc + 1) * TQ],
                                        start=(d == 0), stop=(d == DT - 1),
                                    )
                                res = res_p.tile([P, TQ], F32, tag="res")
                                nc.sync.dma_start(
                                    out=res,
                                    in_=xq[row0 : row0 + P, hc * TQ : (hc + 1) * TQ],
                                )
                                ot = ost_p.tile([P, TQ], F32, tag="ot")
                                nc.vector.tensor_add(out=ot, in0=ps, in1=res)
                                nc.sync.dma_start(
                                    out=out[
                                        row0 : row0 + P, hc * TQ : (hc + 1) * TQ
                                    ],
                                    in_=ot,
                                )

    if compile:
        nc.compile()
    return nc


def _make_masks(j):
    """Additive mask for diagonal tiles: m[u][r, q] = 0 iff r <= GQ*q+j-128u."""
    GQ = NCORES // B
    NDIAG = TQ * GQ // P
    u = np.arange(NDIAG)[:, None, None]
    r = np.arange(P)[None, :, None]
    q = np.arange(TQ)[None, None, :]
    return np.where(r <= GQ * q + j - P * u, 0.0, NEG).astype(np.float32)


_NC_CACHE = None
_last_in_maps = None


def kernel(x, qkv, o_proj):
    global _NC_CACHE
    GQ, SQ, NGRP, HT, DT, NDIAG, ND8 = _derived()
    if _NC_CACHE is None:
        _NC_CACHE = build_nc()
    nc = _NC_CACHE

    x = np.ascontiguousarray(np.asarray(x, dtype=np.float32))
    qkv = np.asarray(qkv, dtype=np.float32)
    o_proj = np.asarray(o_proj, dtype=np.float32)
    wq_h = np.ascontiguousarray(qkv[:, :D]).astype(CDT_NP)
    wk_h = np.ascontiguousarray(qkv[:, D : 2 * D]).astype(CDT_NP)
    wv_h = np.ascontiguousarray(qkv[:, 2 * D :]).astype(CDT_NP)
    wo_h = o_proj.astype(CDT_NP)

    in_maps = []
    for c in range(NCORES):
        b, j = divmod(c, GQ)
        in_maps.append(
            {
                "xkv": np.ascontiguousarray(
                    x[b, j * (S // GQ) : (j + 1) * (S // GQ), :]
                ),
                "xq": np.ascontiguousarray(x[b, j::GQ, :]),
                "wq": wq_h,
                "wk": wk_h,
                "wv": wv_h,
                "wo": wo_h,
                "masks": _make_masks(j),
            }
        )

    global _last_in_maps
    _last_in_maps = in_maps
    res = run_bass_kernel_spmd(nc, in_maps, list(range(NCORES)))

    outp = np.empty((B, S, H), dtype=np.float32)
    for c in range(NCORES):
        b, j = divmod(c, GQ)
        outp[b, j::GQ, :] = res.results[c]["out"]
    return outp



# revision 22
# speedup vs baseline: 1.1332x; 1.1332x over previous
"""Trainium2 Bass kernel for a single-head causal attention block.

Computes, per batch b:
    xn    = LayerNorm(x[b])           (non-affine, eps=1e-6)
    q,k,v = xn @ Wq, xn @ Wk, xn @ Wv
    s     = causal_mask(q @ k.T / sqrt(D))
    out   = softmax(s) @ v @ Wo + x[b]

Key restructuring vs the straightforward dataflow: the only cross-core
data is xn (normalized activations), exchanged with ONE AllGather that
launches right after LayerNorm (~45us in) instead of after the K/V
projections (~400us in).  The K and V projections are reassociated out
of the gathered path:

    s  = (q/sqrt(D)) @ Wk.T @ xn.T     ->  zq := (q/sqrt(D)) @ Wk.T (local)
                                           s  = zq @ xn_full.T
    pv = p @ v = p @ xn @ Wv           ->  y  := p @ xn_full
                                           u  = y @ Wv (local)
    out = u @ Wo + x

Total matmul FLOPs are unchanged, numerics verified within 8e-3 relmax
(vs 7e-3 for the direct dataflow), but every projection becomes local
work that overlaps the collective; the PE never waits ~300us for K/V.

Sharding (8 cores, SPMD single program):
  core c -> batch b = c//4, lane j = c%4.
  Own kv tokens: [1024*j, 1024*(j+1)) (LayerNorm + gather contribution).
  Queries: strided rows {j, j+4, ...} so the causal prefix per query
  group is identical across cores (required: one shared program).
  Group g in {0,1} covers original positions [2048*g, 2048*(g+1)), so
  its visible key prefix is 16*(g+1) key tiles; the last 16 are in the
  causal diagonal band and get additive masks (per-core data).

Matmul orientations (out[M,N] = lhsT[K,M].T @ rhs[K,N], K contracted):
  qT  [d,tq]  : lhsT = Wq[h,d] tile,    rhs = xnT_q[h,tq]  (scaled 1/sqrt(D))
  zqT [h,tq]  : lhsT = WkT[d,h] tile,   rhs = qT[d,tq]
  sT  [tk,tq] : lhsT = xnT[h,tk] tile,  rhs = zqT[h,tq]
  yT  [h,tq]  : lhsT = xn[t,h] tile,    rhs = pT[t,tq]     (p = exp(sT+mask))
  uT  [d,tq]  : lhsT = Wv[h,d] tile,    rhs = yT[h,tq]
  out [tq,h]  : lhsT = uT[d,tq] tile,   rhs = Wo[d,h]
Softmax denominators via an all-ones stationary matmul accumulated over
pT (PSUM, broadcast across partitions); normalization is folded into
the yT PSUM->SBUF eviction.  exp() without max subtraction (scores are
O(10); fp32 PSUM holds exp fine) matches softmax after normalization.

SBUF layout: five 16KB/partition "stash" slots [P,16,TQ]bf16 whose tag
rings are reused across phases (qT0/qT1 -> pT1a/pT1b -> uT0/uT1,
zqT0/zqT1 -> yT0/yT1, pT0), one 64KB weight pool recycled Wq -> WkT ->
Wv -> Wo, and one [P,512]x16x2 fetch pool shared by the q-projection
rhs transposes and the score-phase xn^T fetches.  The tile framework's
WAR tracking on tag reuse provides the phase ordering.
"""

import numpy as np
import ml_dtypes

import concourse.bacc as bacc
import concourse.tile as tile
from concourse import mybir
from concourse.bass_utils import run_bass_kernel_spmd

# Problem shape (hardcoded per harness contract)
B, S, H, D = 2, 4096, 2048, 2048
NCORES = 8
P = 128                 # partitions
TQ = 512                # query group width (matmul moving dim)
GQ = NCORES // B        # cores per batch
SKV = S // GQ           # own kv tokens per core
SQ = S // GQ            # queries per core
NG = SQ // TQ           # query groups per core (2)
HT = H // P             # h chunks (16)
DT = D // P             # d chunks (16)
NDIAG = TQ * GQ // P    # diagonal (masked) key tiles per group (16)

F32 = mybir.dt.float32
BF16 = mybir.dt.bfloat16
CDT_NP = ml_dtypes.bfloat16

EPS = 1e-6
NEG = -1e30


def build_nc(compile=True):
    nc = bacc.Bacc(num_devices=NCORES)

    # I/O
    x_own_bf = nc.dram_tensor("x_own_bf", [SKV, H], BF16, kind="ExternalInput")
    x_q_bf = nc.dram_tensor("x_q_bf", [SQ, H], BF16, kind="ExternalInput")
    x_q = nc.dram_tensor("x_q", [SQ, H], F32, kind="ExternalInput")
    wq = nc.dram_tensor("wq", [H, D], BF16, kind="ExternalInput")
    wkt = nc.dram_tensor("wkt", [D, H], BF16, kind="ExternalInput")
    wv = nc.dram_tensor("wv", [H, D], BF16, kind="ExternalInput")
    wo = nc.dram_tensor("wo", [D, H], BF16, kind="ExternalInput")
    masks = nc.dram_tensor("masks", [NDIAG, P, TQ], BF16, kind="ExternalInput")
    ident = nc.dram_tensor("ident", [P, P], BF16, kind="ExternalInput")
    out = nc.dram_tensor("out", [SQ, H], F32, kind="ExternalOutput")

    # DRAM scratch
    xn_own = nc.dram_tensor("xn_own", [SKV, H], BF16)
    xn_full = nc.dram_tensor("xn_full", [S, H], BF16)
    CC_GROUPS = [list(range(g * GQ, (g + 1) * GQ)) for g in range(B)]

    with (
        tile.TileContext(nc, pool_alloc_mode="queue") as tc,
        tc.tile_pool(name="consts", bufs=1) as consts,
    ):
        ones = consts.tile([P, P], BF16)
        nc.vector.memset(ones, 1.0)
        eps_tile = consts.tile([P, 1], F32)
        nc.vector.memset(eps_tile, EPS)
        id_sb = consts.tile([P, P], BF16)
        nc.sync.dma_start(out=id_sb, in_=ident[:, :])

        def ln_tile(xp, stp, smp, xnp, src, dst, t, alt):
            """LayerNorm tokens [t*P, (t+1)*P) of src -> dst (bf16).

            Normalize alternates DVE / GpSimd so the tiles pipeline at
            bn_stats throughput instead of serializing on DVE.
            """
            x_t = xp.tile([P, H], BF16, tag="x")
            nc.sync.dma_start(out=x_t, in_=src[t * P:(t + 1) * P, :])
            stats = stp.tile([P, H // 512, 6], F32, tag="st")
            for i in range(H // 512):
                nc.vector.bn_stats(
                    out=stats[:, i, :], in_=x_t[:, i * 512:(i + 1) * 512]
                )
            mv = smp.tile([P, 2], F32, tag="mv")
            nc.vector.bn_aggr(out=mv, in_=stats)
            sq = smp.tile([P, 1], F32, tag="sq")
            nc.scalar.activation(
                out=sq, in_=mv[:, 1:2],
                func=mybir.ActivationFunctionType.Sqrt,
                bias=eps_tile, scale=1.0,
            )
            rs = smp.tile([P, 1], F32, tag="rs")
            nc.vector.reciprocal(out=rs, in_=sq)
            xn_t = xnp.tile([P, H], BF16, tag="xn")
            eng = nc.gpsimd if alt else nc.vector
            eng.tensor_scalar(
                out=xn_t, in0=x_t, scalar1=mv[:, 0:1], scalar2=rs,
                op0=mybir.AluOpType.subtract, op1=mybir.AluOpType.mult,
            )
            nc.sync.dma_start(out=dst[t * P:(t + 1) * P, :], in_=xn_t)

        # ---------------- Phase A: LayerNorm, launch gather -------------
        with (
            tc.tile_pool(name="xp", bufs=4) as xp,
            tc.tile_pool(name="stp", bufs=4) as stp,
            tc.tile_pool(name="smp", bufs=8) as smp,
            tc.tile_pool(name="xnp", bufs=4) as xnp,
        ):
            for t in range(SKV // P):
                ln_tile(xp, stp, smp, xnp, x_own_bf, xn_own, t, alt=(t % 2 == 1))

        with (
            tc.tile_pool(name="stash", bufs=1) as stash,
            tc.tile_pool(name="wpool", bufs=1) as wpool,
            tc.tile_pool(name="fpool", bufs=2) as fpool,
            tc.tile_pool(name="stream", bufs=3) as strm,
            tc.tile_pool(name="res", bufs=2) as resp,
            tc.tile_pool(name="ost", bufs=2) as ostp,
            tc.tile_pool(name="rec", bufs=1) as recp,
        ):
            # One AllGather of xn; lands while q/zq projections run.
            # Emitted INSIDE this pool scope: a pool-open boundary makes
            # the pool's first tiles wait on every earlier instruction,
            # which would chain everything behind the ~266us transfer.
            # It must also be the LAST Pool-queue instruction, or later
            # Pool work would publish its engine-sem tick only after the
            # transfer, stalling every cross-engine dependent.
            nc.gpsimd.collective_compute(
                "AllGather", mybir.AluOpType.bypass,
                replica_groups=CC_GROUPS,
                ins=[xn_own[:, :]], outs=[xn_full[:, :]],
            )
            def load_w(w_dram, nm):
                tiles = []
                for a in range(16):
                    t = wpool.tile([P, 2048], BF16, tag=f"w{a}", name=f"{nm}{a}")
                    nc.sync.dma_start(out=t, in_=w_dram[a * P:(a + 1) * P, :])
                    tiles.append(t)
                return tiles

            def slot(idx, nm):
                return stash.tile([P, 16, TQ], BF16, tag=f"U{idx}", name=nm)

            # ------------ Phase C1: q-LayerNorm + Q projection ----------
            # q-LN tiles stage through stash slots (same byte size, re-
            # interpreted shapes); xn^T is built with PE transpose-mode
            # (DmaTransposeAnt would serialize against the in-flight
            # collective), landing in fpool tiles that feed the q matmuls.
            qTs = []
            xnslots = []
            with (
                tc.tile_pool(name="pfa", bufs=4, space="PSUM") as pfa,
                tc.tile_pool(name="ppc", bufs=3, space="PSUM") as ppc,
            ):
                for g in range(NG):
                    xstage = stash.tile(
                        [P, 4, H], BF16, tag=f"U{g}", name=f"xqstage{g}"
                    )
                    xnslot = stash.tile(
                        [P, 4, H], BF16, tag=("U4" if g == 0 else "U2"),
                        name=f"xnq{g}",
                    )
                    xnslots.append(xnslot)
                    for t in range(4):
                        row0 = (g * 4 + t) * P
                        nc.sync.dma_start(
                            out=xstage[:, t, :], in_=x_q_bf[row0:row0 + P, :]
                        )
                        stats = strm.tile(
                            [P, H // 512, 6], F32, tag="st", name=f"stq{g}_{t}"
                        )
                        for k in range(H // 512):
                            nc.vector.bn_stats(
                                out=stats[:, k, :],
                                in_=xstage[:, t, k * 512:(k + 1) * 512],
                            )
                        mv = strm.tile([P, 2], F32, tag="mv", name=f"mv{g}_{t}")
                        nc.vector.bn_aggr(out=mv, in_=stats)
                        sq = strm.tile([P, 1], F32, tag="sq", name=f"sq{g}_{t}")
                        nc.scalar.activation(
                            out=sq, in_=mv[:, 1:2],
                            func=mybir.ActivationFunctionType.Sqrt,
                            bias=eps_tile, scale=1.0,
                        )
                        rs = strm.tile([P, 1], F32, tag="rs", name=f"rs{g}_{t}")
                        nc.vector.reciprocal(out=rs, in_=sq)
                        nc.vector.tensor_scalar(
                            out=xnslot[:, t, :], in0=xstage[:, t, :],
                            scalar1=mv[:, 0:1], scalar2=rs,
                            op0=mybir.AluOpType.subtract,
                            op1=mybir.AluOpType.mult,
                        )
                wq_sb = load_w(wq, "wq")
                for g in range(NG):
                    xnT = []
                    for hc in range(HT):
                        ft = fpool.tile(
                            [P, TQ], BF16, tag=f"f{hc}", name=f"xnT{g}_{hc}"
                        )
                        xnT.append(ft)
                    for hc in range(HT):
                        for t in range(4):
                            pst = pfa.tile(
                                [P, P], BF16, tag="pt", name=f"pt{g}_{hc}_{t}"
                            )
                            nc.tensor.transpose(
                                pst, xnslots[g][:, t, hc * P:(hc + 1) * P], id_sb
                            )
                            nc.any.tensor_copy(
                                xnT[hc][:, t * P:(t + 1) * P], pst
                            )
                    qT = slot(g, f"qT{g}")
                    for dc in range(DT):
                        ps = ppc.tile([P, TQ], F32, tag="ps", name=f"psq{g}_{dc}")
                        for hc in range(HT):
                            nc.tensor.matmul(
                                ps, wq_sb[hc][:, dc * P:(dc + 1) * P], xnT[hc],
                                start=(hc == 0), stop=(hc == HT - 1),
                            )
                        nc.scalar.activation(
                            out=qT[:, dc, :], in_=ps,
                            func=mybir.ActivationFunctionType.Copy,
                            bias=0.0, scale=float(1.0 / np.sqrt(D)),
                        )
                    qTs.append(qT)

            # ------------ Phase C2: zq = (q/sqrt(D)) @ Wk.T -------------
            wkt_sb = load_w(wkt, "wkt")
            zqTs = []
            with tc.tile_pool(name="ppz", bufs=3, space="PSUM") as ppz:
                for g in range(NG):
                    zqT = slot(2 + g, f"zqT{g}")
                    for hh in range(HT):
                        ps = ppz.tile([P, TQ], F32, tag="ps", name=f"psz{g}_{hh}")
                        for dc in range(DT):
                            nc.tensor.matmul(
                                ps, wkt_sb[dc][:, hh * P:(hh + 1) * P],
                                qTs[g][:, dc, :],
                                start=(dc == 0), stop=(dc == DT - 1),
                            )
                        nc.any.tensor_copy(zqT[:, hh, :], ps)
                    zqTs.append(zqT)

            # -------- Phase D: scores, mask, exp, denominators ----------
            # pT storage: group 0 in slot 4; group 1 split over slots 0/1
            # (reusing the dead qT rings).
            pT0 = slot(4, "pT0")
            pT1a = slot(0, "pT1a")
            pT1b = slot(1, "pT1b")

            def pT(g, tk):
                if g == 0:
                    return pT0[:, tk, :]
                return pT1a[:, tk, :] if tk < 16 else pT1b[:, tk - 16, :]

            recs = [
                recp.tile([P, TQ], F32, tag=f"rec{g}", name=f"rec{g}")
                for g in range(NG)
            ]
            with (
                tc.tile_pool(name="psc", bufs=3, space="PSUM") as psc,
                tc.tile_pool(name="psums", bufs=1, space="PSUM") as psm,
            ):
                sums = [
                    psm.tile([P, TQ], F32, tag=f"sums{g}", name=f"sums{g}")
                    for g in range(NG)
                ]
                for kc in range(S // TQ):  # 8 fetches of 4 key tiles
                    xt = []
                    for hc in range(HT):
                        t = fpool.tile(
                            [P, TQ], BF16, tag=f"f{hc}", name=f"f{kc}_{hc}"
                        )
                        nc.scalar.dma_start_transpose(
                            t,
                            xn_full[kc * TQ:(kc + 1) * TQ, hc * P:(hc + 1) * P],
                        )
                        xt.append(t)
                    for t4 in range(4):
                        tk = kc * 4 + t4
                        for g in range(NG):
                            TK = 16 * (g + 1)
                            if tk >= TK:
                                continue
                            ps = psc.tile(
                                [P, TQ], F32, tag="ps", name=f"pss{g}_{tk}"
                            )
                            for hc in range(HT):
                                nc.tensor.matmul(
                                    ps,
                                    xt[hc][:, t4 * P:(t4 + 1) * P],
                                    zqTs[g][:, hc, :],
                                    start=(hc == 0), stop=(hc == HT - 1),
                                )
                            u = tk - (TK - NDIAG)
                            if u >= 0:
                                mt = strm.tile(
                                    [P, TQ], BF16, tag="m", name=f"m{g}_{tk}"
                                )
                                nc.sync.dma_start(out=mt, in_=masks[u, :, :])
                                nc.vector.tensor_add(out=ps, in0=ps, in1=mt)
                            nc.scalar.activation(
                                out=pT(g, tk), in_=ps,
                                func=mybir.ActivationFunctionType.Exp,
                            )
                            nc.tensor.matmul(
                                sums[g], ones, pT(g, tk),
                                start=(tk == 0), stop=(tk == TK - 1),
                                skip_group_check=True,
                            )
                            if tk == TK - 1:
                                nc.vector.reciprocal(out=recs[g], in_=sums[g])

            # ---------- Phase E: yT = (xn_full.T @ pT) * rec ------------
            wv_sb = load_w(wv, "wv")
            yTs = []
            with tc.tile_pool(name="py", bufs=2, space="PSUM") as pyp:
                for g in range(NG):
                    TK = 16 * (g + 1)
                    yT = slot(2 + g, f"yT{g}")
                    for qtr in range(4):
                        pys = [
                            pyp.tile(
                                [P, TQ], F32, tag=f"y{i}",
                                name=f"y{g}_{qtr}_{i}",
                            )
                            for i in range(4)
                        ]
                        for tk in range(TK):
                            xe = strm.tile(
                                [P, 4 * P], BF16, tag="e",
                                name=f"e{g}_{qtr}_{tk}",
                            )
                            nc.sync.dma_start(
                                out=xe,
                                in_=xn_full[
                                    tk * P:(tk + 1) * P,
                                    qtr * TQ:(qtr + 1) * TQ,
                                ],
                            )
                            for i in range(4):
                                nc.tensor.matmul(
                                    pys[i],
                                    xe[:, i * P:(i + 1) * P],
                                    pT(g, tk),
                                    start=(tk == 0), stop=(tk == TK - 1),
                                    skip_group_check=True,
                                )
                        for i in range(4):
                            nc.vector.tensor_mul(
                                out=yT[:, qtr * 4 + i, :],
                                in0=pys[i], in1=recs[g],
                            )
                    yTs.append(yT)

            # ------ Phase F: uT = Wv-contract yT ------------------------
            uTs = []
            with tc.tile_pool(name="ppu", bufs=3, space="PSUM") as ppu:
                for g in range(NG):
                    uT = slot(g, f"uT{g}")
                    for dc in range(DT):
                        ps = ppu.tile([P, TQ], F32, tag="ps", name=f"psu{g}_{dc}")
                        for hc in range(HT):
                            nc.tensor.matmul(
                                ps,
                                wv_sb[hc][:, dc * P:(dc + 1) * P],
                                yTs[g][:, hc, :],
                                start=(hc == 0), stop=(hc == HT - 1),
                            )
                        nc.any.tensor_copy(uT[:, dc, :], ps)
                    uTs.append(uT)

            # -- Phase G: out = uT.T @ Wo + residual ---------------------
            wo_sb = load_w(wo, "wo")
            with tc.tile_pool(name="ppo", bufs=3, space="PSUM") as ppo:
                for g in range(NG):
                    for tq4 in range(TQ // P):
                        row0 = g * TQ + tq4 * P
                        for hs in range(H // TQ):
                            ps = ppo.tile(
                                [P, TQ], F32, tag="ps", name=f"pso{g}_{tq4}_{hs}"
                            )
                            for dc in range(DT):
                                nc.tensor.matmul(
                                    ps,
                                    uTs[g][:, dc, tq4 * P:(tq4 + 1) * P],
                                    wo_sb[dc][:, hs * TQ:(hs + 1) * TQ],
                                    start=(dc == 0), stop=(dc == DT - 1),
                                )
                            res = resp.tile(
                                [P, TQ], F32, tag="r", name=f"r{g}_{tq4}_{hs}"
                            )
                            nc.sync.dma_start(
                                out=res,
                                in_=x_q[row0:row0 + P, hs * TQ:(hs + 1) * TQ],
                            )
                            ot = ostp.tile(
                                [P, TQ], F32, tag="o", name=f"o{g}_{tq4}_{hs}"
                            )
                            nc.vector.tensor_add(out=ot, in0=ps, in1=res)
                            nc.sync.dma_start(
                                out=out[row0:row0 + P, hs * TQ:(hs + 1) * TQ],
                                in_=ot,
                            )

    if compile:
        nc.compile()
    return nc


def _make_masks(j):
    """Additive mask for diagonal tiles: m[u][r, q] = 0 iff r <= GQ*q+j-128u."""
    u = np.arange(NDIAG)[:, None, None]
    r = np.arange(P)[None, :, None]
    q = np.arange(TQ)[None, None, :]
    return np.where(r <= GQ * q + j - P * u, 0.0, NEG).astype(CDT_NP)


def make_in_maps(x, qkv, o_proj):
    x = np.ascontiguousarray(np.asarray(x, dtype=np.float32))
    qkv = np.asarray(qkv, dtype=np.float32)
    o_proj = np.asarray(o_proj, dtype=np.float32)
    wq_h = np.ascontiguousarray(qkv[:, :D]).astype(CDT_NP)
    wkt_h = np.ascontiguousarray(qkv[:, D:2 * D].T).astype(CDT_NP)
    wv_h = np.ascontiguousarray(qkv[:, 2 * D:]).astype(CDT_NP)
    wo_h = o_proj.astype(CDT_NP)

    in_maps = []
    for c in range(NCORES):
        b, j = divmod(c, GQ)
        in_maps.append(
            {
                "x_own_bf": np.ascontiguousarray(
                    x[b, j * SKV:(j + 1) * SKV, :]
                ).astype(CDT_NP),
                "x_q_bf": np.ascontiguousarray(x[b, j::GQ, :]).astype(CDT_NP),
                "x_q": np.ascontiguousarray(x[b, j::GQ, :]),
                "wq": wq_h,
                "wkt": wkt_h,
                "wv": wv_h,
                "wo": wo_h,
                "masks": _make_masks(j),
                "ident": np.eye(P, dtype=CDT_NP),
            }
        )
    return in_maps


_NC_CACHE = None


def kernel(x, qkv, o_proj):
    global _NC_CACHE
    if _NC_CACHE is None:
        _NC_CACHE = build_nc()
    nc = _NC_CACHE

    in_maps = make_in_maps(x, qkv, o_proj)
    res = run_bass_kernel_spmd(nc, in_maps, list(range(NCORES)))

    outp = np.empty((B, S, H), dtype=np.float32)
    for c in range(NCORES):
        b, j = divmod(c, GQ)
        outp[b, j::GQ, :] = res.results[c]["out"]
    return outp
